# revision 12
# baseline (speedup 1.0000x reference)
"""Cross-GNN (3-layer GCN with cross-branch similarity mixing) on 8 trn2 cores.

Sharding: dst nodes across 8 cores; src (node) space padded per-core to
NPAD = 8*6272 so every 128-row src tile belongs to one core. The GCN
aggregation out = D^-1/2 (A+I) D^-1/2 (X W) is computed as dense
per-src-tile tensor-engine matmuls over an fp8 edge-count matrix A
streamed from DRAM:
  psum[32q:32q+32, :] += hs_t^T @ A[t][:, chunk]   (4x column-tiled PE,
                                                    t % 4 == q)
then the four partition bands are reduced with a J = stacked-identity
matmul, scaled by dst-side dinv, and biased. hs (dinv-scaled features,
fp16) lives in SBUF in node-major [128, T*64] layout with per-tile
[branch0 | branch1] slots, so consecutive convs on opposite branches
pipeline without write conflicts. Layer boundaries compute the
similarity mix chunk-locally, transpose to node-major via the PE, and
AllGather 64 features/node; layers 2-3 rebuild hs with DVE-only scaling.
Layer 3 runs both branches in one A_u stream (2x 64-wide column tiles).
"""
import numpy as np
import ml_dtypes

import concourse.bacc as bacc
import concourse.tile as tile
import concourse.bass as bass
from concourse import mybir
from concourse import bass_utils

N = 50000
F_IN = 256
HID = 32
OUT = 128
EPS = 1e-12
N_CORES = 8
SHARD = N // N_CORES          # 6250

DP = 6272                     # padded per-core node count (49*128)
NPAD = DP * N_CORES           # 50176
T = NPAD // 128               # 392 src tiles
TG = 14                       # src tiles per A-DMA group
NTG = T // TG                 # 28
CW = 512                      # dst column chunk width
CHUNKS = [(i * CW, CW) for i in range(DP // CW)] + [(DP - DP % CW, DP % CW)]
assert CHUNKS[-1][1] == 128 and len(CHUNKS) == 13
NJ = T // 4                   # tiles per col-tile position (32-wide conv)

F16 = mybir.dt.float16
F32 = mybir.dt.float32
F8 = mybir.dt.float8e4

_cache = {}


def build_program():
    if "nc" in _cache:
        return _cache["nc"]
    import contextlib

    nc = bacc.Bacc("TRN2", target_bir_lowering=False, debug=False,
                   num_devices=N_CORES, detect_race_conditions=False)

    xT = nc.dram_tensor("xT", [F_IN, NPAD], F16, kind="ExternalInput").ap()
    Win = nc.dram_tensor("Win", [128, 2 * HID], F16, kind="ExternalInput").ap()
    Whid = nc.dram_tensor("Whid", [HID, HID], F16, kind="ExternalInput").ap()
    Wout = nc.dram_tensor("Wout", [HID, OUT], F16, kind="ExternalInput").ap()
    bin_ = nc.dram_tensor("bin", [HID, 1], F32, kind="ExternalInput").ap()
    bhid = nc.dram_tensor("bhid", [HID, 1], F32, kind="ExternalInput").ap()
    bout = nc.dram_tensor("bout", [OUT, 1], F32, kind="ExternalInput").ap()
    dinvTu = nc.dram_tensor("dinvTu", [128, T], F32, kind="ExternalInput").ap()
    dinvTu2 = nc.dram_tensor("dinvTu2", [128, T], F32, kind="ExternalInput").ap()
    dshu = nc.dram_tensor("dshu", [HID, DP], F32, kind="ExternalInput").ap()
    dshu2 = nc.dram_tensor("dshu2", [HID, DP], F32, kind="ExternalInput").ap()
    J4 = nc.dram_tensor("J4", [128, 32], F16, kind="ExternalInput").ap()
    J2m = nc.dram_tensor("J2m", [128, 32], F16, kind="ExternalInput").ap()
    J2s = nc.dram_tensor("J2s", [128, 32], F16, kind="ExternalInput").ap()
    Au = nc.dram_tensor("Au", [T, 128, DP], F8, kind="ExternalInput").ap()
    Au2 = nc.dram_tensor("Au2", [T, 128, DP], F8, kind="ExternalInput").ap()
    y1 = nc.dram_tensor("y1", [OUT, DP], F32, kind="ExternalOutput").ap()
    y2 = nc.dram_tensor("y2", [OUT, DP], F32, kind="ExternalOutput").ap()
    ccin = nc.dram_tensor("ccin", [128, 49 * 64], F16, kind="Internal").ap()
    ccout1 = nc.dram_tensor("ccout1", [N_CORES, 128, 49 * 64], F16,
                            kind="Internal", addr_space="Shared").ap()
    ccout2 = nc.dram_tensor("ccout2", [N_CORES, 128, 49 * 64], F16,
                            kind="Internal", addr_space="Shared").ap()

    with tile.TileContext(nc) as tc:
        ctx = contextlib.ExitStack()
        with ctx:
            persist = ctx.enter_context(tc.tile_pool(name="persist", bufs=1))
            stream = ctx.enter_context(tc.tile_pool(name="stream", bufs=2))
            astream = ctx.enter_context(tc.tile_pool(name="astream", bufs=3))
            psA = ctx.enter_context(tc.tile_pool(name="psA", bufs=2,
                                                 space="PSUM"))
            psJ = ctx.enter_context(tc.tile_pool(name="psJ", bufs=2,
                                                 space="PSUM"))
            pstr = ctx.enter_context(tc.tile_pool(name="pstr", bufs=1,
                                                  space="PSUM"))
            psaux = ctx.enter_context(tc.tile_pool(name="psaux", bufs=2,
                                                   space="PSUM"))

            hs_all = persist.tile([128, T * 64], F16, tag="hs_all")
            x1t = persist.tile([32, DP], F16, tag="x1t")
            x2t = persist.tile([32, DP], F16, tag="x2t")
            dshu_sb = persist.tile([32, DP], F32, tag="dshu")
            dshu2_sb = persist.tile([32, DP], F32, tag="dshu2")
            dinvTu_sb = persist.tile([128, T], F32, tag="dinvTu")
            dinvTu2_sb = persist.tile([128, T], F32, tag="dinvTu2")
            Win_sb = persist.tile([128, 2 * HID], F16, tag="winsb")
            Whid_sb = persist.tile([32, HID], F16, tag="whidsb")
            Wout_sb = persist.tile([32, OUT], F16, tag="woutsb")
            bin_sb = persist.tile([HID, 1], F32, tag="binsb")
            bhid_sb = persist.tile([HID, 1], F32, tag="bhidsb")
            bout_sb = persist.tile([OUT, 1], F32, tag="boutsb")
            J4_sb = persist.tile([128, 32], F16, tag="j4")
            J2m_sb = persist.tile([128, 32], F16, tag="j2m")
            J2s_sb = persist.tile([128, 32], F16, tag="j2s")
            ones32c = persist.tile([32, 1], F16, tag="ones32c")
            ones32r = persist.tile([1, 32], F16, tag="ones32r")
            ccin_sb = persist.tile([128, 49 * 64], F16, tag="ccin_sb")

            for dst, src in ((Win_sb, Win), (Whid_sb, Whid), (Wout_sb, Wout),
                             (bin_sb, bin_), (bhid_sb, bhid), (bout_sb, bout),
                             (dinvTu_sb, dinvTu), (dinvTu2_sb, dinvTu2),
                             (dshu_sb, dshu), (dshu2_sb, dshu2),
                             (J4_sb, J4), (J2m_sb, J2m), (J2s_sb, J2s)):
                nc.sync.dma_start(dst[:], src[:])
            nc.vector.memset(ones32c[:], 1.0)
            nc.vector.memset(ones32r[:], 1.0)

            def build_l1():
                """hs_all <- dinv * (X @ Win) for both branches (one XW)."""
                for b in range(T // 4):
                    xt = stream.tile([128, 2 * CW], F16, tag="xt")
                    nc.sync.dma_start(xt[:, 0:CW], xT[0:128, b * CW:(b + 1) * CW])
                    nc.sync.dma_start(xt[:, CW:2 * CW],
                                      xT[128:256, b * CW:(b + 1) * CW])
                    for i in range(4):
                        t = b * 4 + i
                        ps = pstr.tile([128, 32], F32, tag="l1ps")
                        for k in range(2):
                            nc.tensor.matmul(
                                out=ps[:],
                                lhsT=xt[:, k * CW + i * 128:k * CW + (i + 1) * 128],
                                rhs=Win_sb[:, k * HID:(k + 1) * HID],
                                start=(k == 0), stop=(k == 1))
                        nc.vector.tensor_scalar_mul(
                            hs_all[:, t * 64:t * 64 + 32], ps[:],
                            dinvTu_sb[:, t:t + 1])
                        nc.vector.tensor_scalar_mul(
                            hs_all[:, t * 64 + 32:t * 64 + 64], ps[:],
                            dinvTu2_sb[:, t:t + 1])

            def build_l23(ccout, dinvA, dinvB):
                """hs_all <- dinv * allgathered node-major features."""
                fused = dinvA is dinvB
                for c in range(N_CORES):
                    tab = stream.tile([128, 49 * 64], F16, tag="tab")
                    nc.sync.dma_start(tab[:], ccout[c, :, :])
                    for j in range(49):
                        t = c * 49 + j
                        if fused:
                            nc.vector.tensor_scalar_mul(
                                hs_all[:, t * 64:t * 64 + 64],
                                tab[:, j * 64:j * 64 + 64], dinvA[:, t:t + 1])
                        else:
                            nc.vector.tensor_scalar_mul(
                                hs_all[:, t * 64:t * 64 + 32],
                                tab[:, j * 64:j * 64 + 32], dinvA[:, t:t + 1])
                            nc.vector.tensor_scalar_mul(
                                hs_all[:, t * 64 + 32:t * 64 + 64],
                                tab[:, j * 64 + 32:j * 64 + 64],
                                dinvB[:, t:t + 1])

            def conv32(A_ap, br, b_sb, out_t, dsh):
                """out_t[32, DP] = dsh * (hs_br^T @ A) + b  (4x col-tiled)."""
                for (c0, cw) in CHUNKS:
                    psa = psA.tile([128, CW], F32, tag="psa")
                    for g in range(NTG):
                        sbA = astream.tile([128, TG * CW], F8, tag="sbA")
                        nc.sync.dma_start(
                            sbA[:, :TG * cw].rearrange("p (t c) -> p t c", c=cw),
                            A_ap[g * TG:(g + 1) * TG, :,
                                 c0:c0 + cw].transpose([1, 0, 2]))
                        for j in range(TG):
                            t = g * TG + j
                            q = t % 4
                            nc.tensor.matmul(
                                out=psa[32 * q:32 * q + 32, :cw],
                                lhsT=hs_all[:, t * 64 + br * 32:
                                            t * 64 + br * 32 + 32],
                                rhs=sbA[:, j * cw:(j + 1) * cw],
                                start=(t < 4), stop=(t >= T - 4),
                                tile_position=(0, 32 * q))
                    scr = stream.tile([128, CW], F16, tag="scr")
                    nc.scalar.copy(scr[:, :cw], psa[:, :cw])
                    psj = psJ.tile([32, CW], F32, tag="psj")
                    nc.tensor.matmul(out=psj[:, :cw], lhsT=J4_sb[:],
                                     rhs=scr[:, :cw], start=True, stop=True)
                    sl = slice(c0, c0 + cw)
                    scr2 = stream.tile([32, CW], F32, tag="scr2")
                    nc.vector.tensor_tensor(out=scr2[:, :cw], in0=psj[:, :cw],
                                            in1=dsh[:, sl],
                                            op=mybir.AluOpType.mult)
                    nc.vector.tensor_scalar_add(out_t[:, sl], scr2[:, :cw],
                                                b_sb[:])

            def conv64(A_ap):
                """x1t, x2t = dsh_u * (hs^T @ A_u) for both branches at once."""
                for (c0, cw) in CHUNKS:
                    psa = psA.tile([128, CW], F32, tag="psa")
                    for g in range(NTG):
                        sbA = astream.tile([128, TG * CW], F8, tag="sbA")
                        nc.sync.dma_start(
                            sbA[:, :TG * cw].rearrange("p (t c) -> p t c", c=cw),
                            A_ap[g * TG:(g + 1) * TG, :,
                                 c0:c0 + cw].transpose([1, 0, 2]))
                        for j in range(TG):
                            t = g * TG + j
                            q = t % 2
                            nc.tensor.matmul(
                                out=psa[64 * q:64 * q + 64, :cw],
                                lhsT=hs_all[:, t * 64:(t + 1) * 64],
                                rhs=sbA[:, j * cw:(j + 1) * cw],
                                start=(t < 2), stop=(t >= T - 2),
                                tile_position=(0, 64 * q))
                    scr = stream.tile([128, CW], F16, tag="scr")
                    nc.scalar.copy(scr[:, :cw], psa[:, :cw])
                    sl = slice(c0, c0 + cw)
                    for (Jm, out_t) in ((J2m_sb, x1t), (J2s_sb, x2t)):
                        psj = psJ.tile([32, CW], F32, tag="psj")
                        nc.tensor.matmul(out=psj[:, :cw], lhsT=Jm[:],
                                         rhs=scr[:, :cw], start=True, stop=True)
                        scr2 = stream.tile([32, CW], F32, tag="scr2")
                        nc.vector.tensor_tensor(out=scr2[:, :cw],
                                                in0=psj[:, :cw],
                                                in1=dshu_sb[:, sl],
                                                op=mybir.AluOpType.mult)
                        nc.vector.tensor_copy(out_t[:, sl], scr2[:, :cw])

            def boundary(with_W, ccout):
                """x1t,x2t -> similarity mix (m,s), optionally *Whid,
                transpose to node-major, allgather into ccout."""
                for ci, (c0, cw) in enumerate(CHUNKS):
                    sl = slice(c0, c0 + cw)
                    rows = []
                    for (a, b) in ((x1t, x1t), (x2t, x2t), (x1t, x2t)):
                        prod = stream.tile([32, CW], F16, tag="prod")
                        nc.vector.tensor_tensor(out=prod[:, :cw],
                                                in0=a[:, sl], in1=b[:, sl],
                                                op=mybir.AluOpType.mult)
                        pss = psaux.tile([1, CW], F32, tag="aux")
                        nc.tensor.matmul(out=pss[:, :cw], lhsT=ones32c[:],
                                         rhs=prod[:, :cw], start=True,
                                         stop=True)
                        row = stream.tile([1, CW], F32, tag=f"row{len(rows)}")
                        nc.scalar.copy(row[:, :cw], pss[:, :cw])
                        rows.append(row)
                    n1, n2, dot = rows
                    for r in (n1, n2):
                        nc.scalar.sqrt(r[:, :cw], r[:, :cw])
                        nc.vector.tensor_scalar_max(r[:, :cw], r[:, :cw], EPS)
                        nc.vector.reciprocal(r[:, :cw], r[:, :cw])
                    nc.vector.tensor_tensor(out=dot[:, :cw], in0=dot[:, :cw],
                                            in1=n1[:, :cw],
                                            op=mybir.AluOpType.mult)
                    nc.vector.tensor_tensor(out=dot[:, :cw], in0=dot[:, :cw],
                                            in1=n2[:, :cw],
                                            op=mybir.AluOpType.mult)
                    simv = stream.tile([1, CW], F16, tag="simv")
                    nc.vector.tensor_copy(simv[:, :cw], dot[:, :cw])
                    psrep = psaux.tile([32, CW], F32, tag="aux")
                    nc.tensor.matmul(out=psrep[:, :cw], lhsT=ones32r[:],
                                     rhs=simv[:, :cw], start=True, stop=True)
                    simrep = stream.tile([32, CW], F16, tag="simrep")
                    nc.scalar.copy(simrep[:, :cw], psrep[:, :cw])
                    # m = x1 + x2*sim ; s = x2 + x1*sim  (chunk-local, in place)
                    t1 = stream.tile([32, CW], F16, tag="t1")
                    t2 = stream.tile([32, CW], F16, tag="t2")
                    nc.vector.tensor_tensor(out=t1[:, :cw], in0=x2t[:, sl],
                                            in1=simrep[:, :cw],
                                            op=mybir.AluOpType.mult)
                    nc.vector.tensor_tensor(out=t2[:, :cw], in0=x1t[:, sl],
                                            in1=simrep[:, :cw],
                                            op=mybir.AluOpType.mult)
                    nc.vector.tensor_tensor(out=x1t[:, sl], in0=x1t[:, sl],
                                            in1=t1[:, :cw],
                                            op=mybir.AluOpType.add)
                    nc.vector.tensor_tensor(out=x2t[:, sl], in0=x2t[:, sl],
                                            in1=t2[:, :cw],
                                            op=mybir.AluOpType.add)
                    if with_W:
                        for zt in (x1t, x2t):
                            psw = psaux.tile([32, CW], F32, tag="aux")
                            nc.tensor.matmul(out=psw[:, :cw], lhsT=Whid_sb[:],
                                             rhs=zt[:, sl], start=True,
                                             stop=True)
                            nc.scalar.copy(zt[:, sl], psw[:, :cw])
                # transpose to node-major [128, 49*64] via xbar DMA
                ccv = ccin_sb[:].rearrange("p (j g) -> p j g", g=64)
                nc.sync.dma_start_transpose(ccv[:, :, 0:32], x1t[:])
                nc.sync.dma_start_transpose(ccv[:, :, 32:64], x2t[:])
                nc.sync.dma_start(ccin[:], ccin_sb[:])
                nc.gpsimd.collective_compute(
                    "AllGather", mybir.AluOpType.bypass,
                    replica_groups=[list(range(N_CORES))],
                    ins=[ccin[:]], outs=[ccout[:]])

            def final_out(zf, y_ap):
                for (c0, cw) in CHUNKS:
                    sl = slice(c0, c0 + cw)
                    psy = psA.tile([128, CW], F32, tag="psa")
                    nc.tensor.matmul(out=psy[:, :cw], lhsT=Wout_sb[:],
                                     rhs=zf[:, sl], start=True, stop=True)
                    yst = stream.tile([128, CW], F32, tag="yst")
                    nc.vector.tensor_scalar_add(yst[:, :cw], psy[:, :cw],
                                                bout_sb[:])
                    nc.sync.dma_start(y_ap[:, sl], yst[:, :cw])

            # ================= schedule =================
            build_l1()
            conv32(Au, 0, bin_sb, x1t, dshu_sb)
            conv32(Au2, 1, bin_sb, x2t, dshu2_sb)
            boundary(True, ccout1)
            build_l23(ccout1, dinvTu_sb, dinvTu2_sb)
            conv32(Au, 0, bhid_sb, x1t, dshu_sb)
            conv32(Au2, 1, bhid_sb, x2t, dshu2_sb)
            boundary(False, ccout2)
            build_l23(ccout2, dinvTu_sb, dinvTu_sb)
            conv64(Au)
            final_out(x1t, y1)
            final_out(x2t, y2)

    nc.compile()
    _cache["nc"] = nc
    return nc


def _prep_inputs(x, ei_u, ei_u2, W_in, W_hid, W_out, b_in, b_hid, b_out):
    f8lut = np.arange(64, dtype=np.float32).astype(ml_dtypes.float8_e4m3)
    nodes = np.arange(N, dtype=np.int64)
    pad_idx = nodes + (DP - SHARD) * (nodes // SHARD)  # padded-global index

    xT16 = np.zeros((F_IN, NPAD), np.float16)
    xT16[:, pad_idx] = np.asarray(x, np.float32).T

    def dinv_of(ei):
        d = np.bincount(np.asarray(ei[1], np.int64), minlength=N)
        return 1.0 / np.sqrt((d + 1.0).astype(np.float32))

    dinv_u = dinv_of(ei_u)
    dinv_u2 = dinv_of(ei_u2)

    def dinvT_layout(dinv):
        arr = np.ones(NPAD, np.float32)
        arr[pad_idx] = dinv
        return np.ascontiguousarray(arr.reshape(T, 128).T)

    def dsh_layout(dinv, c):
        row = np.ones(DP, np.float32)
        row[:SHARD] = dinv[c * SHARD:(c + 1) * SHARD]
        return np.tile(row[None, :], (HID, 1))

    def build_A(ei, c):
        lo, hi = c * SHARD, (c + 1) * SHARD
        src = np.asarray(ei[0], np.int64)
        dst = np.asarray(ei[1], np.int64)
        m = (dst >= lo) & (dst < hi)
        s = pad_idx[src[m]]
        d = dst[m] - lo
        buf = np.zeros(NPAD * DP, np.uint8)
        np.add.at(buf, s * DP + d, 1)
        g = np.arange(lo, hi, dtype=np.int64)
        buf[pad_idx[g] * DP + (g - lo)] += 1
        return f8lut[np.minimum(buf, 63)].reshape(T, 128, DP)

    p = np.arange(128)
    J4 = (p[:, None] % 32 == np.arange(32)[None, :]).astype(np.float16)
    J2m = (p[:, None] % 64 == np.arange(32)[None, :]).astype(np.float16)
    J2s = (p[:, None] % 64 == np.arange(32)[None, :] + 32).astype(np.float16)

    common = {
        "xT": xT16,
        "Win": np.concatenate([np.asarray(W_in, np.float32)[:128],
                               np.asarray(W_in, np.float32)[128:]],
                              axis=1).astype(np.float16),
        "Whid": np.asarray(W_hid, np.float32).astype(np.float16),
        "Wout": np.asarray(W_out, np.float32).astype(np.float16),
        "bin": np.asarray(b_in, np.float32).reshape(HID, 1),
        "bhid": np.asarray(b_hid, np.float32).reshape(HID, 1),
        "bout": np.asarray(b_out, np.float32).reshape(OUT, 1),
        "dinvTu": dinvT_layout(dinv_u), "dinvTu2": dinvT_layout(dinv_u2),
        "J4": J4, "J2m": J2m, "J2s": J2s,
    }
    per_core = []
    for c in range(N_CORES):
        im = dict(common)
        im["dshu"] = dsh_layout(dinv_u, c)
        im["dshu2"] = dsh_layout(dinv_u2, c)
        im["Au"] = build_A(ei_u, c)
        im["Au2"] = build_A(ei_u2, c)
        per_core.append(im)
    return per_core


LAST_RESULT = None


def kernel(x, edge_index_u, edge_index_u2, W_in, b_in, W_hid, b_hid,
           W_out, b_out):
    global LAST_RESULT
    nc = build_program()
    in_maps = _prep_inputs(x, edge_index_u, edge_index_u2,
                           W_in, W_hid, W_out, b_in, b_hid, b_out)
    res = bass_utils.run_bass_kernel_spmd(nc, in_maps,
                                          core_ids=list(range(N_CORES)))
    LAST_RESULT = res
    out = np.zeros((N, 2 * OUT), np.float32)
    for c in range(N_CORES):
        lo, hi = c * SHARD, (c + 1) * SHARD
        out[lo:hi, 0:OUT] = res.results[c]["y1"][:, :SHARD].T
        out[lo:hi, OUT:2 * OUT] = res.results[c]["y2"][:, :SHARD].T
    return out
